# revision 9
# baseline (speedup 1.0000x reference)
"""Trainium2 Bass kernel for nn_Basic_Operator_59365037965641.

out = w0*(x+y) + w1*x*y + w2*x/(|y|+eps) + w3*y/(|x|+eps)
    + w4*x*sin(y) + w5*y*sin(x),   w = softmax(param,0).sum(1)

Factored: out = x*A(y) + y*B(x),
    A(y) = w0 + w1*y + w2*g(y) + w4*sin(y),   g(t) = 1/(|t|+eps)
    B(x) = w0 + w3*g(x) + w5*sin(x)

x,y column-slices are concatenated into one [128, 4096] tile per
iteration (32 iterations/core). Engine split per iteration:
  DVE : u = range-wrap into [-pi,pi]          (custom ADD_RANGE_WRAP)
        g = 1/(|t|+eps)                       (custom ABS_EPS_RECIP_1NR:
             abs + eps + bitwise-NOT seed + one recentered Newton step,
             8/8 DVE stages, ~0.17% max rel err)
        o = p1 + p2 -> bf16                   (tensor_tensor add, 2x_1p)
  ACT : s = Sin(u); evac psA/psB + w0 -> bf16 ([1024] slabs)
  PE  : psA = w1*y + w2*gy + w4*sy ; psB = w3*gx + w5*sx
        (all f32r diag matmuls - weights exact)
  POOL: p1 = A_sb*x ; p2 = B_sb*y             (tensor_tensor mult)
  DMA : f32 in, bf16 out (halved write traffic; rel-err budget 2e-2)

Data-parallel across 8 cores on the leading dim of x/y (flattened rows).
"""

import os
import sys

import numpy as np

sys.path.insert(0, "/opt/trn_rl_repo")

from contextlib import ExitStack

import concourse.bass as bass
import concourse.tile as tile
from concourse import bacc, mybir

EPS = 1e-8
PI = float(np.pi)
TWO_PI = float(2.0 * np.pi)
# 1-NR reciprocal constants: Chebyshev seed scale (imm2) and recentered
# Newton constant (s1) from RECIP_APPROX_FAST_CONSTS.
RC_SEED = -0.23549792
RC_NR = 2.0017324

N_CORES = 8
FULL_ROWS = 16384            # 4*4096
COLS = 4096
SHARD_ROWS = FULL_ROWS // N_CORES       # 2048
P = 128
F = 2048                     # output cols per iteration
CF = 2 * F                   # concat width (x-half | y-half)
ROW_TILES = SHARD_ROWS // P             # 16
COL_TILES = COLS // F                   # 2
SLAB = 1024                  # psum slab cols
CHUNK = 512                  # matmul moving-dim chunk
# final-add column split: [0:ADD_W] on DVE, [ADD_W:F] on Pool
ADD_W = int(os.environ.get("KADDW", "1536"))

f32 = mybir.dt.float32
f32r = mybir.dt.float32r
bf16 = mybir.dt.bfloat16
Alu = mybir.AluOpType
Act = mybir.ActivationFunctionType

_cached = {}


def _register_abs_eps_recip():
    import concourse.dve_ops as D
    from concourse.dve_ops import DveOp, Spec
    from concourse.dve_spec import Src0, C0, C1, C2, maxx, Zero
    import re

    name = "ABS_EPS_RECIP_1NR"
    if name in D._SUB_OPCODE_FOR_NAME:
        return [o for o in D.OPS if o.name == name][0]

    _neg = Zero - Src0
    _ax = maxx(Src0, _neg) + C0
    _nx = D.Bin(D.AluOp.BITWISE_NOT, _ax, _ax)
    _y0 = _nx * C2
    body = _y0 * (C1 - _ax * _y0)

    def ref(in0, in1, c0, c1, c2):
        ax = (np.maximum(in0, -in0) + c0).astype(np.float32)
        nx = (~ax.view(np.int32)).view(np.float32)
        y0 = nx * np.float32(c2)
        return y0 * (np.float32(c1) - ax * y0)

    op = DveOp(name, Spec(body=body, reference=ref), subdim=False, uops_sha={})
    D.OPS.append(op)
    D._SUB_OPCODE_FOR_NAME[op.name] = D._CUSTOM_DVE_ROW_BASE + len(D.OPS) - 1
    D.CUSTOM_DVE_SPECS[op.name] = op.spec
    for ver in ("v3", "v4"):
        try:
            op.compile(ver)
        except ValueError as e:
            m = re.search(rf"{ver}: ([0-9a-f]+)", str(e))
            if m:
                op.uops_sha[ver] = m.group(1)
            else:
                raise
    op.compile("v3")
    return op


def build_bass(w0):
    """Only w0 is baked into instructions (ACT evac bias); w1..w5 arrive
    exact via the f32r diags input."""
    from concourse.dve_ops import ADD_RANGE_WRAP

    op_aer = _register_abs_eps_recip()

    nc = bacc.Bacc("TRN2", target_bir_lowering=False, debug=False)

    x_d = nc.dram_tensor("x", [SHARD_ROWS, COLS], f32, kind="ExternalInput")
    y_d = nc.dram_tensor("y", [SHARD_ROWS, COLS], f32, kind="ExternalInput")
    # 5 stacked [128,128] diagonal matrices: w1, w2, w4, w3, w5
    dg_d = nc.dram_tensor("diags", [P, 5 * P], f32, kind="ExternalInput")
    out_d = nc.dram_tensor("out", [SHARD_ROWS, COLS], bf16, kind="ExternalOutput")

    xv = x_d.ap().rearrange("(n p) c -> n p c", p=P)   # [16, 128, 4096]
    yv = y_d.ap().rearrange("(n p) c -> n p c", p=P)
    ov = out_d.ap().rearrange("(n p) c -> n p c", p=P)

    with tile.TileContext(nc) as tc, ExitStack() as ctx:
        const_pool = ctx.enter_context(tc.tile_pool(name="const", bufs=1))
        io_pool = ctx.enter_context(tc.tile_pool(name="io", bufs=4))
        u_pool = ctx.enter_context(tc.tile_pool(name="u", bufs=3))
        g_pool = ctx.enter_context(tc.tile_pool(name="g", bufs=3))
        s_pool = ctx.enter_context(tc.tile_pool(name="s", bufs=3))
        ab_pool = ctx.enter_context(tc.tile_pool(name="ab", bufs=3))
        p_pool = ctx.enter_context(tc.tile_pool(name="pp", bufs=3))
        o_pool = ctx.enter_context(tc.tile_pool(name="o", bufs=3))
        ps_pool = ctx.enter_context(tc.tile_pool(name="ps", bufs=4, space="PSUM"))

        diags = const_pool.tile([P, 5 * P], f32r)
        nc.sync.dma_start(diags[:], dg_d.ap().bitcast(f32r))
        d_w1 = diags[:, 0 * P : 1 * P]
        diagsb = const_pool.tile([P, 4 * P], bf16)
        nc.vector.tensor_copy(diagsb[:], diags[:, P:].bitcast(f32))
        d_w2 = diagsb[:, 0 * P : 1 * P]
        d_w4 = diagsb[:, 1 * P : 2 * P]
        d_w3 = diagsb[:, 2 * P : 3 * P]
        d_w5 = diagsb[:, 3 * P : 4 * P]

        def emit_add(pend):
            p1, p2, r0, csl0 = pend
            o_t = o_pool.tile([P, F], bf16, tag="o")
            if ADD_W >= F:
                nc.vector.tensor_tensor(o_t[:], p1[:], p2[:], Alu.add)
            elif ADD_W <= 0:
                nc.gpsimd.tensor_tensor(o_t[:], p1[:], p2[:], Alu.add)
            else:
                nc.vector.tensor_tensor(o_t[:, :ADD_W], p1[:, :ADD_W],
                                        p2[:, :ADD_W], Alu.add)
                nc.gpsimd.tensor_tensor(o_t[:, ADD_W:], p1[:, ADD_W:],
                                        p2[:, ADD_W:], Alu.add)
            nc.scalar.dma_start(ov[r0][:, csl0], o_t[:])

        pending = None
        for r in range(ROW_TILES):
            for cidx in range(COL_TILES):
                csl = slice(cidx * F, (cidx + 1) * F)
                io = io_pool.tile([P, CF], f32r, tag="io")
                nc.sync.dma_start(io[:, :F], xv[r][:, csl].bitcast(f32r))
                nc.sync.dma_start(io[:, F:], yv[r][:, csl].bitcast(f32r))
                io_f = io[:].bitcast(f32)

                # --- DVE: range-wrap + fused abs/eps/reciprocal ---
                u = u_pool.tile([P, CF], bf16, tag="u")
                nc.vector.add_range_wrap(u[:], io_f, 0.0, PI, TWO_PI)
                g = g_pool.tile([P, CF], bf16, tag="g")
                nc.vector._custom_dve(op_aer, out=g[:], in0=io_f,
                                      s0=EPS, s1=RC_NR, imm2=RC_SEED)

                # --- ACT: sin over both halves ---
                s = s_pool.tile([P, CF], bf16, tag="s")
                nc.scalar.activation(s[:], u[:], Act.Sin)

                # --- PE sums; ACT evac (+w0); Pool products ---
                ab = ab_pool.tile([P, CF], bf16, tag="ab")  # A_sb | B_sb
                p1 = p_pool.tile([P, F], bf16, tag="p1")
                p2 = p_pool.tile([P, F], bf16, tag="p2")
                for sidx in range(F // SLAB):
                    for half, (p_t, off) in enumerate(((p1, F), (p2, 0))):
                        # half 0: psA from y-half inputs, times x-half
                        # half 1: psB from x-half inputs, times y-half
                        ps = ps_pool.tile([P, SLAB], f32, tag="ps")
                        for c in range(SLAB // CHUNK):
                            pcs = slice(c * CHUNK, (c + 1) * CHUNK)
                            lo = off + sidx * SLAB + c * CHUNK
                            cs = slice(lo, lo + CHUNK)
                            if half == 0:
                                nc.tensor.matmul(ps[:, pcs], d_w1, io[:, cs],
                                                 start=True, stop=False)
                                nc.tensor.matmul(ps[:, pcs], d_w2, g[:, cs],
                                                 start=False, stop=False)
                                nc.tensor.matmul(ps[:, pcs], d_w4, s[:, cs],
                                                 start=False, stop=True)
                            else:
                                nc.tensor.matmul(ps[:, pcs], d_w3, g[:, cs],
                                                 start=True, stop=False)
                                nc.tensor.matmul(ps[:, pcs], d_w5, s[:, cs],
                                                 start=False, stop=True)
                        asl = slice(half * F + sidx * SLAB,
                                    half * F + (sidx + 1) * SLAB)
                        nc.scalar.activation(ab[:, asl], ps[:], Act.Copy,
                                             bias=w0, scale=1.0)
                        ssl = slice(sidx * SLAB, (sidx + 1) * SLAB)
                        msl = slice(F - off + sidx * SLAB,
                                    F - off + (sidx + 1) * SLAB)
                        nc.gpsimd.tensor_tensor(p_t[:, ssl], ab[:, asl],
                                                io_f[:, msl], Alu.mult)

                # --- final add for the PREVIOUS iteration (sw pipeline) ---
                if pending is not None:
                    emit_add(pending)
                pending = (p1, p2, r, csl)

        emit_add(pending)

    nc.finalize()
    return nc


def _get_program(w0):
    key = float(np.float32(w0))
    if key not in _cached:
        _cached[key] = build_bass(key)
    return _cached[key]


def _weights(param):
    param = np.asarray(param, dtype=np.float64)
    m = param.max(axis=0, keepdims=True)
    e = np.exp(param - m)
    soft = e / e.sum(axis=0, keepdims=True)
    return soft.sum(axis=1)  # [6]


def _diags(w):
    eye = np.eye(P, dtype=np.float32)
    order = [w[1], w[2], w[4], w[3], w[5]]
    return np.concatenate([eye * np.float32(v) for v in order], axis=1).astype(
        np.float32
    )


def _run(x, y, param, trace=False):
    from concourse.bass_utils import run_bass_kernel_spmd

    x = np.asarray(x)
    y = np.asarray(y)
    w = _weights(param)
    nc = _get_program(w[0])

    xf = np.ascontiguousarray(x.reshape(FULL_ROWS, COLS))
    yf = np.ascontiguousarray(y.reshape(FULL_ROWS, COLS))
    dg = _diags(w)

    in_maps = []
    for c in range(N_CORES):
        rows = slice(c * SHARD_ROWS, (c + 1) * SHARD_ROWS)
        in_maps.append({"x": xf[rows], "y": yf[rows], "diags": dg})

    res = run_bass_kernel_spmd(
        nc, in_maps, core_ids=list(range(N_CORES)), trace=trace
    )
    out = np.empty((FULL_ROWS, COLS), dtype=np.float32)
    for c in range(N_CORES):
        out[c * SHARD_ROWS : (c + 1) * SHARD_ROWS] = np.asarray(
            res.results[c]["out"]
        ).astype(np.float32)
    return out.reshape(x.shape), res


def kernel(x, y, param):
    out, _ = _run(x, y, param, trace=False)
    return out


def kernel_traced(x, y, param):
    """Run with tracing; returns exec_time_ns (or None)."""
    out, res = _run(x, y, param, trace=True)
    return res.exec_time_ns


# revision 11
# speedup vs baseline: 1.1597x; 1.1597x over previous
"""Trainium2 Bass kernel for nn_Basic_Operator_59365037965641.

out = w0*(x+y) + w1*x*y + w2*x/(|y|+eps) + w3*y/(|x|+eps)
    + w4*x*sin(y) + w5*y*sin(x),   w = softmax(param,0).sum(1)

Factored: out = x*A(y) + y*B(x),
    A(y) = w0 + w1*y + w2*g(y) + w4*sin(y),   g(t) = 1/(|t|+eps)
    B(x) = w0 + w3*g(x) + w5*sin(x)

x,y column-slices are concatenated into one [128, 4096] tile per
iteration (32 iterations/core). Engine split per iteration:
  DVE : u = range-wrap into [-pi,pi]          (custom ADD_RANGE_WRAP)
        g = 1/(|t|+eps)                       (custom ABS_EPS_RECIP_1NR:
             abs + eps + bitwise-NOT seed + one recentered Newton step,
             8/8 DVE stages, ~0.17% max rel err)
        o = p1 + p2 -> bf16                   (tensor_tensor add, 2x_1p)
  ACT : s = Sin(u); evac psA/psB + w0 -> bf16 ([1024] slabs)
  PE  : psA = w1*y + w2*gy + w4*sy ; psB = w3*gx + w5*sx
        (all f32r diag matmuls - weights exact)
  POOL: p1 = A_sb*x ; p2 = B_sb*y             (tensor_tensor mult)
  DMA : f32 in, bf16 out (halved write traffic; rel-err budget 2e-2)

Data-parallel across 8 cores on the leading dim of x/y (flattened rows).
"""

import os
import sys

import numpy as np

sys.path.insert(0, "/opt/trn_rl_repo")

from contextlib import ExitStack

import concourse.bass as bass
import concourse.tile as tile
from concourse import bacc, mybir

EPS = 1e-8
PI = float(np.pi)
TWO_PI = float(2.0 * np.pi)
# 1-NR reciprocal constants: Chebyshev seed scale (imm2) and recentered
# Newton constant (s1) from RECIP_APPROX_FAST_CONSTS.
RC_SEED = -0.23549792
RC_NR = 2.0017324

N_CORES = 8
FULL_ROWS = 16384            # 4*4096
COLS = 4096
SHARD_ROWS = FULL_ROWS // N_CORES       # 2048
P = 128
F = 2048                     # output cols per iteration
CF = 2 * F                   # concat width (x-half | y-half)
ROW_TILES = SHARD_ROWS // P             # 16
COL_TILES = COLS // F                   # 2
SLAB = 1024                  # psum slab cols
CHUNK = 512                  # matmul moving-dim chunk
# final-add column split: [0:ADD_W] on DVE, [ADD_W:F] on Pool
ADD_W = int(os.environ.get("KADDW", "1536"))
DEFER = int(os.environ.get("KDEFER", "2"))

f32 = mybir.dt.float32
f32r = mybir.dt.float32r
bf16 = mybir.dt.bfloat16
Alu = mybir.AluOpType
Act = mybir.ActivationFunctionType

_cached = {}


def _register_abs_eps_recip():
    import concourse.dve_ops as D
    from concourse.dve_ops import DveOp, Spec
    from concourse.dve_spec import Src0, C0, C1, C2, maxx, Zero
    import re

    name = "ABS_EPS_RECIP_1NR"
    if name in D._SUB_OPCODE_FOR_NAME:
        return [o for o in D.OPS if o.name == name][0]

    _neg = Zero - Src0
    _ax = maxx(Src0, _neg) + C0
    _nx = D.Bin(D.AluOp.BITWISE_NOT, _ax, _ax)
    _y0 = _nx * C2
    body = _y0 * (C1 - _ax * _y0)

    def ref(in0, in1, c0, c1, c2):
        ax = (np.maximum(in0, -in0) + c0).astype(np.float32)
        nx = (~ax.view(np.int32)).view(np.float32)
        y0 = nx * np.float32(c2)
        return y0 * (np.float32(c1) - ax * y0)

    op = DveOp(name, Spec(body=body, reference=ref), subdim=False, uops_sha={})
    D.OPS.append(op)
    D._SUB_OPCODE_FOR_NAME[op.name] = D._CUSTOM_DVE_ROW_BASE + len(D.OPS) - 1
    D.CUSTOM_DVE_SPECS[op.name] = op.spec
    for ver in ("v3", "v4"):
        try:
            op.compile(ver)
        except ValueError as e:
            m = re.search(rf"{ver}: ([0-9a-f]+)", str(e))
            if m:
                op.uops_sha[ver] = m.group(1)
            else:
                raise
    op.compile("v3")
    return op


def build_bass(w0):
    """Only w0 is baked into instructions (ACT evac bias); w1..w5 arrive
    exact via the f32r diags input."""
    from concourse.dve_ops import ADD_RANGE_WRAP

    op_aer = _register_abs_eps_recip()

    nc = bacc.Bacc("TRN2", target_bir_lowering=False, debug=False)

    x_d = nc.dram_tensor("x", [SHARD_ROWS, COLS], f32, kind="ExternalInput")
    y_d = nc.dram_tensor("y", [SHARD_ROWS, COLS], f32, kind="ExternalInput")
    # 5 stacked [128,128] diagonal matrices: w1, w2, w4, w3, w5
    dg_d = nc.dram_tensor("diags", [P, 5 * P], f32, kind="ExternalInput")
    out_d = nc.dram_tensor("out", [SHARD_ROWS, COLS], bf16, kind="ExternalOutput")

    xv = x_d.ap().rearrange("(n p) c -> n p c", p=P)   # [16, 128, 4096]
    yv = y_d.ap().rearrange("(n p) c -> n p c", p=P)
    ov = out_d.ap().rearrange("(n p) c -> n p c", p=P)

    with tile.TileContext(nc) as tc, ExitStack() as ctx:
        const_pool = ctx.enter_context(tc.tile_pool(name="const", bufs=1))
        io_pool = ctx.enter_context(tc.tile_pool(name="io", bufs=4))
        u_pool = ctx.enter_context(tc.tile_pool(name="u", bufs=3))
        g_pool = ctx.enter_context(tc.tile_pool(name="g", bufs=3))
        s_pool = ctx.enter_context(tc.tile_pool(name="s", bufs=3))
        ab_pool = ctx.enter_context(tc.tile_pool(name="ab", bufs=3))
        p_pool = ctx.enter_context(tc.tile_pool(name="pp", bufs=3))
        o_pool = ctx.enter_context(tc.tile_pool(name="o", bufs=3))
        ps_pool = ctx.enter_context(tc.tile_pool(name="ps", bufs=4, space="PSUM"))

        diags = const_pool.tile([P, 5 * P], f32r)
        nc.sync.dma_start(diags[:], dg_d.ap().bitcast(f32r))
        d_w1 = diags[:, 0 * P : 1 * P]
        diagsb = const_pool.tile([P, 4 * P], bf16)
        nc.vector.tensor_copy(diagsb[:], diags[:, P:].bitcast(f32))
        d_w2 = diagsb[:, 0 * P : 1 * P]
        d_w4 = diagsb[:, 1 * P : 2 * P]
        d_w3 = diagsb[:, 2 * P : 3 * P]
        d_w5 = diagsb[:, 3 * P : 4 * P]

        def emit_add(pend):
            p1, p2, r0, csl0 = pend
            o_t = o_pool.tile([P, F], bf16, tag="o")
            if ADD_W >= F:
                nc.vector.tensor_tensor(o_t[:], p1[:], p2[:], Alu.add)
            elif ADD_W <= 0:
                nc.gpsimd.tensor_tensor(o_t[:], p1[:], p2[:], Alu.add)
            else:
                nc.vector.tensor_tensor(o_t[:, :ADD_W], p1[:, :ADD_W],
                                        p2[:, :ADD_W], Alu.add)
                nc.gpsimd.tensor_tensor(o_t[:, ADD_W:], p1[:, ADD_W:],
                                        p2[:, ADD_W:], Alu.add)
            nc.scalar.dma_start(ov[r0][:, csl0], o_t[:])

        pending = []
        for r in range(ROW_TILES):
            for cidx in range(COL_TILES):
                csl = slice(cidx * F, (cidx + 1) * F)
                io = io_pool.tile([P, CF], f32r, tag="io")
                nc.sync.dma_start(io[:, :F], xv[r][:, csl].bitcast(f32r))
                nc.sync.dma_start(io[:, F:], yv[r][:, csl].bitcast(f32r))
                io_f = io[:].bitcast(f32)

                # --- DVE: range-wrap + fused abs/eps/reciprocal ---
                u = u_pool.tile([P, CF], bf16, tag="u")
                nc.vector.add_range_wrap(u[:], io_f, 0.0, PI, TWO_PI)
                g = g_pool.tile([P, CF], bf16, tag="g")
                nc.vector._custom_dve(op_aer, out=g[:], in0=io_f,
                                      s0=EPS, s1=RC_NR, imm2=RC_SEED)

                # --- ACT: sin over both halves ---
                s = s_pool.tile([P, CF], bf16, tag="s")
                nc.scalar.activation(s[:], u[:], Act.Sin)

                # --- PE sums; ACT evac (+w0); Pool products ---
                ab = ab_pool.tile([P, CF], bf16, tag="ab")  # A_sb | B_sb
                p1 = p_pool.tile([P, F], bf16, tag="p1")
                p2 = p_pool.tile([P, F], bf16, tag="p2")
                for half, (p_t, off) in enumerate(((p1, F), (p2, 0))):
                    # half 0: psA from y-half inputs, multiplied by x-half
                    # half 1: psB from x-half inputs, multiplied by y-half
                    for sidx in range(F // SLAB):
                        ps = ps_pool.tile([P, SLAB], f32, tag="ps")
                        for c in range(SLAB // CHUNK):
                            pcs = slice(c * CHUNK, (c + 1) * CHUNK)
                            lo = off + sidx * SLAB + c * CHUNK
                            cs = slice(lo, lo + CHUNK)
                            if half == 0:
                                nc.tensor.matmul(ps[:, pcs], d_w1, io[:, cs],
                                                 start=True, stop=False)
                                nc.tensor.matmul(ps[:, pcs], d_w2, g[:, cs],
                                                 start=False, stop=False)
                                nc.tensor.matmul(ps[:, pcs], d_w4, s[:, cs],
                                                 start=False, stop=True)
                            else:
                                nc.tensor.matmul(ps[:, pcs], d_w3, g[:, cs],
                                                 start=True, stop=False)
                                nc.tensor.matmul(ps[:, pcs], d_w5, s[:, cs],
                                                 start=False, stop=True)
                        asl = slice(half * F + sidx * SLAB,
                                    half * F + (sidx + 1) * SLAB)
                        nc.scalar.activation(ab[:, asl], ps[:], Act.Copy,
                                             bias=w0, scale=1.0)
                    # multiply by the OTHER half of io
                    hsl = slice(half * F, (half + 1) * F)
                    msl = slice(F - off, 2 * F - off)
                    nc.gpsimd.tensor_tensor(p_t[:], ab[:, hsl], io_f[:, msl],
                                            Alu.mult)

                # --- final add deferred 2 iterations (sw pipeline) ---
                pending.append((p1, p2, r, csl))
                if len(pending) > DEFER:
                    emit_add(pending.pop(0))

        for pend in pending:
            emit_add(pend)

    nc.finalize()
    return nc


def _get_program(w0):
    key = float(np.float32(w0))
    if key not in _cached:
        _cached[key] = build_bass(key)
    return _cached[key]


def _weights(param):
    param = np.asarray(param, dtype=np.float64)
    m = param.max(axis=0, keepdims=True)
    e = np.exp(param - m)
    soft = e / e.sum(axis=0, keepdims=True)
    return soft.sum(axis=1)  # [6]


def _diags(w):
    eye = np.eye(P, dtype=np.float32)
    order = [w[1], w[2], w[4], w[3], w[5]]
    return np.concatenate([eye * np.float32(v) for v in order], axis=1).astype(
        np.float32
    )


def _run(x, y, param, trace=False):
    from concourse.bass_utils import run_bass_kernel_spmd

    x = np.asarray(x)
    y = np.asarray(y)
    w = _weights(param)
    nc = _get_program(w[0])

    xf = np.ascontiguousarray(x.reshape(FULL_ROWS, COLS))
    yf = np.ascontiguousarray(y.reshape(FULL_ROWS, COLS))
    dg = _diags(w)

    in_maps = []
    for c in range(N_CORES):
        rows = slice(c * SHARD_ROWS, (c + 1) * SHARD_ROWS)
        in_maps.append({"x": xf[rows], "y": yf[rows], "diags": dg})

    res = run_bass_kernel_spmd(
        nc, in_maps, core_ids=list(range(N_CORES)), trace=trace
    )
    out = np.empty((FULL_ROWS, COLS), dtype=np.float32)
    for c in range(N_CORES):
        out[c * SHARD_ROWS : (c + 1) * SHARD_ROWS] = np.asarray(
            res.results[c]["out"]
        ).astype(np.float32)
    return out.reshape(x.shape), res


def kernel(x, y, param):
    out, _ = _run(x, y, param, trace=False)
    return out


def kernel_traced(x, y, param):
    """Run with tracing; returns exec_time_ns (or None)."""
    out, res = _run(x, y, param, trace=True)
    return res.exec_time_ns
